# revision 27
# baseline (speedup 1.0000x reference)
"""BERT self-attention (B=4, L=2048, H=1024, 16 heads) on 8 trn2 NeuronCores.

Sharding: core c = (g, b) with b = batch index (4) and g = head-half (2).
Each core computes Q/K/V projections for its 8 heads over its batch, then
full attention for those heads, producing out[b, :, g*512:(g+1)*512].

On-core layout is "transposed": x arrives pre-transposed from the host
(x^T[hidden, token]), projections produce q^T / k^T with head-dim on
partitions, scores are computed transposed (s^T[key, query]) so softmax'd
probabilities land directly in the layout the P@V matmul needs (keys on the
contraction/partition dim) — no O(L^2) transposes. Softmax skips the max
subtraction (scores ~ N(0,1): exp is safe in fp32) and the normalization is
deferred: V is augmented with a constant ones column (memset once) so each
P@V matmul also yields the exp-sum row, and the division happens once on the
[64, 512] output tile via a fast approximate reciprocal.

Scores matmuls for the two heads of a pair contract over disjoint partition
halves (rows 0-63 / 64-127) and are emitted adjacently so the PE runs them
concurrently in different row groups. A fraction of the exp work can be
offloaded from the Scalar engine to the Vector engine via a one-instruction
Schraudolph-style fast exp that emits float16 bit patterns directly
(DVE_KCP below), balancing the two engines.
"""

import contextlib
import os
import sys

for _p in ("/opt/trn_rl_repo",):
    if os.path.isdir(_p) and _p not in sys.path:
        sys.path.insert(0, _p)

import numpy as np

import concourse.bass as bass
import concourse.tile as tile
from concourse import bacc, mybir
from concourse.bass_utils import run_bass_kernel_spmd

F32 = mybir.dt.float32
F32R = mybir.dt.float32r
F16 = mybir.dt.float16
I16 = mybir.dt.int16
AF = mybir.ActivationFunctionType
ALU = mybir.AluOpType

B, L, HIDDEN = 4, 2048, 1024
NH, D = 16, 64
N_CORES = 8
GDIM = 512            # output dims per core (8 heads x 64)
PAIRS = 4             # head pairs per core (2 heads share a 128-partition group)
TCH = 4               # token chunks of 512
HCH = 8               # hidden chunks of 128
VSTRIDE = 66          # per-head stride in vaug: 64 dims + 1 ones col + 1 pad
VAUG = 8 * VSTRIDE    # 528 cols per 128-token chunk

# kcp indices (of 8) whose exp runs on the Vector engine (fast-exp) instead
# of the Scalar engine. () disables the offload.
DVE_KCP = ()
# fast-exp constants: f16 bits of exp(s/8) ~= int16(A2*s + B2)
EXP_A2 = 0.125 * 1.4426950408889634 * 1024.0
EXP_B2 = 15.0 * 1024.0 - 44.0   # magic offset tuned for min rel error

_NC_CACHE = {}


def _build(fast_mask: bool, has_bqk: bool, has_bv: bool, repeat: int = 1):
    EDT = F16
    nc = bacc.Bacc("TRN2", target_bir_lowering=False, debug=False)
    x_d = nc.dram_tensor("xT", [HIDDEN, L], F32R, kind="ExternalInput")
    wq_d = nc.dram_tensor("wqT", [HIDDEN, GDIM], F32R, kind="ExternalInput")
    wk_d = nc.dram_tensor("wkT", [HIDDEN, GDIM], F32R, kind="ExternalInput")
    wv_d = nc.dram_tensor("wvT", [HIDDEN, GDIM], F32R, kind="ExternalInput")
    bq_d = nc.dram_tensor("bq", [GDIM], F32, kind="ExternalInput")
    bk_d = nc.dram_tensor("bk", [GDIM], F32, kind="ExternalInput")
    bvA_d = nc.dram_tensor("bvA", [64, PAIRS], F32, kind="ExternalInput")
    bvB_d = nc.dram_tensor("bvB", [64, PAIRS], F32, kind="ExternalInput")
    mb_d = nc.dram_tensor("maskb", [L], F32, kind="ExternalInput")
    ones_d = nc.dram_tensor("ones", [1, 128], F32R, kind="ExternalInput")
    out_d = nc.dram_tensor("out", [GDIM, L], F32, kind="ExternalOutput")

    with nc.allow_low_precision(reason="fp32r attention"), tile.TileContext(nc) as tc:
        with (
            tc.tile_pool(name="consts", bufs=1) as consts,
            tc.tile_pool(name="qkv", bufs=1) as qkv,
        ):
            bq_sb = consts.tile([128, PAIRS], F32)
            bk_sb = consts.tile([128, PAIRS], F32)
            bvA_sb = consts.tile([64, PAIRS], F32)
            bvB_sb = consts.tile([64, PAIRS], F32)
            mb_sb = consts.tile([128, 16], F32)
            if has_bqk:
                nc.sync.dma_start(bq_sb[:], bq_d.rearrange("(c p) -> p c", p=128))
                nc.sync.dma_start(bk_sb[:], bk_d.rearrange("(c p) -> p c", p=128))
            if has_bv:
                nc.sync.dma_start(bvA_sb[:], bvA_d[:])
                nc.sync.dma_start(bvB_sb[:], bvB_d[:])
            if not fast_mask:
                nc.sync.dma_start(mb_sb[:], mb_d.rearrange("(c p) -> p c", p=128))

            # persistent per-core projections
            q_sb = qkv.tile([128, PAIRS, L], F32R)      # q^T: [dim-in-pair, pair, token]
            k_sb = qkv.tile([128, PAIRS, L], F32R)
            vaug_sb = qkv.tile([128, L // 128, VAUG], EDT)  # [token-in-chunk, chunk, headcol]
            # constant ones columns for the deferred-softmax sum rows
            for h in range(8):
                nc.vector.memset(vaug_sb[:, :, h * VSTRIDE + 64:h * VSTRIDE + 65], 1.0)
            # loop-invariant weights; split per hidden-chunk so the first
            # projection chain starts as soon as its first slices land
            wq_sb = qkv.tile([128, HCH, GDIM], F32R)
            wk_sb = qkv.tile([128, HCH, GDIM], F32R)
            wv_sb = qkv.tile([128, HCH, GDIM], F32R)
            wqT_all = wq_d.rearrange("(c p) m -> p c m", p=128)
            wkT_all = wk_d.rearrange("(c p) m -> p c m", p=128)
            for hc in range(HCH):
                nc.sync.dma_start(wq_sb[:, hc, :], wqT_all[:, hc, :])
                nc.sync.dma_start(wk_sb[:, hc, :], wkT_all[:, hc, :])
            nc.sync.dma_start(wv_sb[:], wv_d.rearrange("(c p) m -> p c m", p=128))

            def _emit_body():
                # ---------------- phase 1: Q/K projections ----------------
                # V projections are deferred into early phase 2 (below) where
                # they act as dependency-light PE filler that keeps the array
                # dense (and the clock un-throttled) through the phase
                # boundary and attention-pipeline ramp-up.
                xT_all = x_d.rearrange("(c p) t -> p c t", p=128)
                with (
                    tc.tile_pool(name="xt", bufs=2) as xtp,
                    tc.tile_pool(name="projps", bufs=2, space="PSUM") as projps,
                ):
                    for tci in range(TCH):
                        t0 = tci * 512
                        xt = xtp.tile([128, HCH, 512], F32R, tag="xt")
                        for hc in range(HCH):
                            nc.sync.dma_start(
                                xt[:, hc, :], xT_all[:, hc, t0:t0 + 512]
                            )
                        # q^T / k^T for each pair (dc), this token chunk
                        for dc in range(PAIRS):
                            qp = projps.tile([128, 512], F32, tag="qps")
                            kp = projps.tile([128, 512], F32, tag="kps")
                            for hc in range(HCH):
                                nc.tensor.matmul(
                                    qp[:], wq_sb[:, hc, dc * 128:(dc + 1) * 128],
                                    xt[:, hc, :],
                                    start=(hc == 0), stop=(hc == HCH - 1),
                                )
                            for hc in range(HCH):
                                nc.tensor.matmul(
                                    kp[:], wk_sb[:, hc, dc * 128:(dc + 1) * 128],
                                    xt[:, hc, :],
                                    start=(hc == 0), stop=(hc == HCH - 1),
                                )
                            if has_bqk:
                                nc.vector.tensor_scalar_add(
                                    q_sb[:, dc, t0:t0 + 512], qp[:], bq_sb[:, dc:dc + 1]
                                )
                                nc.vector.tensor_scalar_add(
                                    k_sb[:, dc, t0:t0 + 512], kp[:], bk_sb[:, dc:dc + 1]
                                )
                            else:
                                nc.vector.tensor_copy(q_sb[:, dc, t0:t0 + 512], qp[:])
                                nc.vector.tensor_copy(k_sb[:, dc, t0:t0 + 512], kp[:])

                # ---------------- phase 2: attention ----------------
                # Software-pipelined emission: scores are emitted two units
                # ahead of the exp/PV that consume them, so the tensor queue
                # never head-blocks on the ACT engine and the PE stays dense.
                with (
                    tc.tile_pool(name="epool", bufs=4) as epool,
                    tc.tile_pool(name="obuf", bufs=2) as obuf,
                    tc.tile_pool(name="xv", bufs=2) as xvp,
                    tc.tile_pool(name="scps", bufs=2, space="PSUM") as scps,
                    tc.tile_pool(name="ops", bufs=2, space="PSUM") as opsp,
                ):
                    def emit_vchains(cpair):
                        # V projection for key chunks 2*cpair, 2*cpair+1:
                        # woven into early attention as PE filler. PSUM comes
                        # from the scores ring (short-lived).
                        c0 = cpair * 2
                        xv = xvp.tile([128, HCH, 256], F32R, tag="xv")
                        nc.sync.dma_start(
                            xv[:], xT_all[:, :, c0 * 128:(c0 + 2) * 128]
                        )
                        for ci in range(2):
                            vp = scps.tile([128, 512], F32, tag="s")
                            for hc in range(HCH):
                                nc.tensor.matmul(
                                    vp[:], xv[:, hc, ci * 128:(ci + 1) * 128],
                                    wv_sb[:, hc, :],
                                    start=(hc == 0), stop=(hc == HCH - 1),
                                )
                            nc.vector.tensor_copy(
                                vaug_sb[:, c0 + ci, :].rearrange(
                                    "p (h s) -> p h s", h=8
                                )[:, :, 0:64],
                                vp[:].rearrange("p (h d) -> p h d", h=8),
                            )
                    KSTEPS = 8 if fast_mask else 16
                    NKC = 2 if fast_mask else 1    # key chunks per unit
                    SFREE = 512 * NKC
                    units = [
                        (p, qc, ks)
                        for p in range(PAIRS)
                        for qc in range(TCH)
                        for ks in range(KSTEPS)
                    ]
                    state = {}

                    def emit_scores(u):
                        p, qc, ks = u
                        q0 = qc * 512
                        sA = scps.tile([128, SFREE], F32, tag="s")
                        sB = scps.tile([128, SFREE], F32, tag="s")
                        for j in range(NKC):
                            kc = ks * NKC + j
                            # adjacent emission: rows 0-63 / 64-127 run
                            # concurrently in different row groups
                            nc.tensor.matmul(
                                sA[:, j * 512:(j + 1) * 512],
                                k_sb[0:64, p, kc * 128:(kc + 1) * 128],
                                q_sb[0:64, p, q0:q0 + 512],
                                start=True, stop=True,
                            )
                            nc.tensor.matmul(
                                sB[:, j * 512:(j + 1) * 512],
                                k_sb[64:128, p, kc * 128:(kc + 1) * 128],
                                q_sb[64:128, p, q0:q0 + 512],
                                start=True, stop=True,
                            )
                        state[u] = (sA, sB)

                    def emit_exp_pv(u, oA, oB):
                        p, qc, ks = u
                        hA, hB = 2 * p, 2 * p + 1
                        cA, cB = hA * VSTRIDE, hB * VSTRIDE
                        sA, sB = state.pop(u)
                        if ks in DVE_KCP and fast_mask:
                            eiA = epool.tile([128, SFREE], I16, tag="e")
                            eiB = epool.tile([128, SFREE], I16, tag="e")
                            nc.vector.tensor_scalar(
                                eiA[:], sA[:], EXP_A2, EXP_B2,
                                op0=ALU.mult, op1=ALU.add,
                            )
                            nc.vector.tensor_scalar(
                                eiB[:], sB[:], EXP_A2, EXP_B2,
                                op0=ALU.mult, op1=ALU.add,
                            )
                            eA = eiA.bitcast(F16)
                            eB = eiB.bitcast(F16)
                        else:
                            eA = epool.tile([128, SFREE], EDT, tag="e")
                            eB = epool.tile([128, SFREE], EDT, tag="e")
                            if fast_mask:
                                nc.scalar.activation(eA[:], sA[:], AF.Exp, scale=0.125)
                                nc.scalar.activation(eB[:], sB[:], AF.Exp, scale=0.125)
                            else:
                                kc = ks
                                nc.scalar.activation(
                                    eA[:], sA[:], AF.Exp,
                                    bias=mb_sb[:, kc:kc + 1], scale=0.125,
                                )
                                nc.scalar.activation(
                                    eB[:], sB[:], AF.Exp,
                                    bias=mb_sb[:, kc:kc + 1], scale=0.125,
                                )
                        for j in range(NKC):
                            kc = ks * NKC + j
                            first = kc == 0
                            last = kc == L // 128 - 1
                            nc.tensor.matmul(
                                oA[:], vaug_sb[:, kc, cA:cA + 65],
                                eA[:, j * 512:(j + 1) * 512],
                                start=first, stop=last,
                            )
                            nc.tensor.matmul(
                                oB[:], vaug_sb[:, kc, cB:cB + 65],
                                eB[:, j * 512:(j + 1) * 512],
                                start=first, stop=last,
                            )

                    def emit_norm(p, qc, oA, oB):
                        hA, hB = 2 * p, 2 * p + 1
                        q0 = qc * 512
                        denA = obuf.tile([1, 512], F32, tag="denA")
                        denB = obuf.tile([1, 512], F32, tag="denB")
                        nc.vector.tensor_copy(denA[:], oA[64:65, :])
                        nc.vector.tensor_copy(denB[:], oB[64:65, :])
                        recA = obuf.tile([1, 512], F32, tag="recA")
                        recB = obuf.tile([1, 512], F32, tag="recB")
                        nc.vector.reciprocal_approx_fast(recA[:], denA[:])
                        nc.vector.reciprocal_approx_fast(recB[:], denB[:])
                        for idx, (o_ps, rec, h, bv_sb) in enumerate((
                            (oA, recA, hA, bvA_sb),
                            (oB, recB, hB, bvB_sb),
                        )):
                            bc_sb = obuf.tile([64, 512], F32, tag=f"bcsb{idx}")
                            nc.gpsimd.partition_broadcast(bc_sb[:], rec[:])
                            o_sb = obuf.tile([64, 512], F32, tag=f"osb{idx}")
                            nc.vector.tensor_tensor(
                                out=o_sb[:], in0=o_ps[0:64, :], in1=bc_sb[:],
                                op=ALU.mult,
                            )
                            if has_bv:
                                nc.vector.tensor_scalar_add(
                                    o_sb[:], o_sb[:], bv_sb[:, p:p + 1]
                                )
                            d0 = p * 128 + (h % 2) * 64
                            nc.sync.dma_start(
                                out_d[d0:d0 + 64, q0:q0 + 512], o_sb[:]
                            )

                    def emit_scores_at(j):
                        # weave the deferred V projections ahead of the first
                        # few units' scores
                        if fast_mask:
                            if j < 8:
                                emit_vchains(j)
                        else:
                            if j < 16 and j % 2 == 0:
                                emit_vchains(j // 2)
                        emit_scores(units[j])

                    emit_scores_at(0)
                    emit_scores_at(1)
                    o_cur = None
                    for i, u in enumerate(units):
                        p, qc, ks = u
                        if ks == 0:
                            oA = opsp.tile([65, 512], F32, tag="oA")
                            oB = opsp.tile([65, 512], F32, tag="oB")
                            o_cur = (oA, oB)
                        # PV first: it unblocks on exp(i) which completes
                        # before scores(i+2)'s ring slot frees, so the PE
                        # never head-blocks on a later dependency.
                        emit_exp_pv(u, *o_cur)
                        if i + 2 < len(units):
                            emit_scores_at(i + 2)
                        if ks == KSTEPS - 1:
                            emit_norm(p, qc, *o_cur)

            loop_cm = (
                tc.For_i(0, repeat, 1) if repeat > 1 else contextlib.nullcontext()
            )
            with loop_cm:
                _emit_body()

    nc.finalize()
    return nc


def _get_nc(fast_mask: bool, has_bqk: bool, has_bv: bool):
    key = (fast_mask, has_bqk, has_bv)
    if key not in _NC_CACHE:
        _NC_CACHE[key] = _build(*key)
    return _NC_CACHE[key]


def _prep_in_maps(x, masked_attention, Wq, bq, Wk, bk, Wv, bv):
    x = np.asarray(x, np.float32)
    mask = np.asarray(masked_attention, np.float32)
    Wq = np.asarray(Wq, np.float32)
    Wk = np.asarray(Wk, np.float32)
    Wv = np.asarray(Wv, np.float32)
    bq = np.asarray(bq, np.float32)
    bk = np.asarray(bk, np.float32)
    bv = np.asarray(bv, np.float32)

    ones = np.ones((1, 128), np.float32)
    maskb = (mask - 1.0) * 10000.0

    per_g = []
    for g in range(2):
        sl = slice(g * GDIM, (g + 1) * GDIM)
        wqT = np.ascontiguousarray(Wq[sl, :].T)
        wkT = np.ascontiguousarray(Wk[sl, :].T)
        wvT = np.ascontiguousarray(Wv[sl, :].T)
        bq_g = bq[sl].copy()
        bk_g = bk[sl].copy()
        bv_g = bv[sl].reshape(8, 64)
        bvA = np.ascontiguousarray(bv_g[0::2].T)  # [64, PAIRS]
        bvB = np.ascontiguousarray(bv_g[1::2].T)
        per_g.append((wqT, wkT, wvT, bq_g, bk_g, bvA, bvB))

    xT = [np.ascontiguousarray(x[b].T) for b in range(B)]

    in_maps = []
    for c in range(N_CORES):
        g, b = divmod(c, B)
        wqT, wkT, wvT, bq_g, bk_g, bvA, bvB = per_g[g]
        in_maps.append({
            "xT": xT[b],
            "wqT": wqT, "wkT": wkT, "wvT": wvT,
            "bq": bq_g, "bk": bk_g, "bvA": bvA, "bvB": bvB,
            "maskb": np.ascontiguousarray(maskb[b]),
            "ones": ones,
        })

    fast_mask = bool(np.all(mask == 1.0))
    has_bqk = bool(np.any(bq) or np.any(bk))
    has_bv = bool(np.any(bv))
    return in_maps, fast_mask, has_bqk, has_bv


def _gather(results):
    out = np.empty((B, L, HIDDEN), np.float32)
    for c in range(N_CORES):
        g, b = divmod(c, B)
        out[b, :, g * GDIM:(g + 1) * GDIM] = results[c]["out"].T
    return out


def kernel(x, masked_attention, Wq, bq, Wk, bk, Wv, bv):
    in_maps, fast_mask, has_bqk, has_bv = _prep_in_maps(
        x, masked_attention, Wq, bq, Wk, bk, Wv, bv
    )
    nc = _get_nc(fast_mask, has_bqk, has_bv)
    res = run_bass_kernel_spmd(nc, in_maps, core_ids=list(range(N_CORES)))
    return _gather(res.results)


# revision 29
# speedup vs baseline: 1.1840x; 1.1840x over previous
"""BERT self-attention (B=4, L=2048, H=1024, 16 heads) on 8 trn2 NeuronCores.

Sharding: core c = (g, b) with b = batch index (4) and g = head-half (2).
Each core computes Q/K/V projections for its 8 heads over its batch, then
full attention for those heads, producing out[b, :, g*512:(g+1)*512].

On-core layout is "transposed": x arrives pre-transposed from the host
(x^T[hidden, token]), projections produce q^T / k^T with head-dim on
partitions, scores are computed transposed (s^T[key, query]) so softmax'd
probabilities land directly in the layout the P@V matmul needs (keys on the
contraction/partition dim) — no O(L^2) transposes. Softmax skips the max
subtraction (scores ~ N(0,1): exp is safe in fp32) and the normalization is
deferred: V is augmented with a constant ones column (memset once) so each
P@V matmul also yields the exp-sum row, and the division happens once on the
[64, 512] output tile via a fast approximate reciprocal.

Scores matmuls for the two heads of a pair contract over disjoint partition
halves (rows 0-63 / 64-127) and are emitted adjacently so the PE runs them
concurrently in different row groups. A fraction of the exp work can be
offloaded from the Scalar engine to the Vector engine via a one-instruction
Schraudolph-style fast exp that emits float16 bit patterns directly
(DVE_KCP below), balancing the two engines.
"""

import contextlib
import os
import sys

for _p in ("/opt/trn_rl_repo",):
    if os.path.isdir(_p) and _p not in sys.path:
        sys.path.insert(0, _p)

import numpy as np

import concourse.bass as bass
import concourse.tile as tile
from concourse import bacc, mybir
from concourse.bass_utils import run_bass_kernel_spmd

F32 = mybir.dt.float32
F32R = mybir.dt.float32r
F16 = mybir.dt.float16
I16 = mybir.dt.int16
AF = mybir.ActivationFunctionType
ALU = mybir.AluOpType

B, L, HIDDEN = 4, 2048, 1024
NH, D = 16, 64
N_CORES = 8
GDIM = 512            # output dims per core (8 heads x 64)
PAIRS = 4             # head pairs per core (2 heads share a 128-partition group)
TCH = 4               # token chunks of 512
HCH = 8               # hidden chunks of 128
VSTRIDE = 66          # per-head stride in vaug: 64 dims + 1 ones col + 1 pad
VAUG = 8 * VSTRIDE    # 528 cols per 128-token chunk

# kcp indices (of 8) whose exp runs on the Vector engine (fast-exp) instead
# of the Scalar engine. () disables the offload.
DVE_KCP = ()
# fast-exp constants: f16 bits of exp(s/8) ~= int16(A2*s + B2)
EXP_A2 = 0.125 * 1.4426950408889634 * 1024.0
EXP_B2 = 15.0 * 1024.0 - 44.0   # magic offset tuned for min rel error

_NC_CACHE = {}


def _build(fast_mask: bool, has_bqk: bool, has_bv: bool, repeat: int = 1):
    EDT = F16
    nc = bacc.Bacc("TRN2", target_bir_lowering=False, debug=False)
    x_d = nc.dram_tensor("xT", [HIDDEN, L], F32R, kind="ExternalInput")
    wq_d = nc.dram_tensor("wqT", [HIDDEN, GDIM], F32R, kind="ExternalInput")
    wk_d = nc.dram_tensor("wkT", [HIDDEN, GDIM], F32R, kind="ExternalInput")
    wv_d = nc.dram_tensor("wvT", [HIDDEN, GDIM], F32R, kind="ExternalInput")
    bq_d = nc.dram_tensor("bq", [GDIM], F32, kind="ExternalInput")
    bk_d = nc.dram_tensor("bk", [GDIM], F32, kind="ExternalInput")
    bvA_d = nc.dram_tensor("bvA", [64, PAIRS], F32, kind="ExternalInput")
    bvB_d = nc.dram_tensor("bvB", [64, PAIRS], F32, kind="ExternalInput")
    mb_d = nc.dram_tensor("maskb", [L], F32, kind="ExternalInput")
    ones_d = nc.dram_tensor("ones", [1, 128], F32R, kind="ExternalInput")
    out_d = nc.dram_tensor("out", [GDIM, L], F32, kind="ExternalOutput")

    with nc.allow_low_precision(reason="fp32r attention"), tile.TileContext(nc) as tc:
        with (
            tc.tile_pool(name="consts", bufs=1) as consts,
            tc.tile_pool(name="qkv", bufs=1) as qkv,
        ):
            bq_sb = consts.tile([128, PAIRS], F32)
            bk_sb = consts.tile([128, PAIRS], F32)
            bvA_sb = consts.tile([64, PAIRS], F32)
            bvB_sb = consts.tile([64, PAIRS], F32)
            mb_sb = consts.tile([128, 16], F32)
            if has_bqk:
                nc.sync.dma_start(bq_sb[:], bq_d.rearrange("(c p) -> p c", p=128))
                nc.sync.dma_start(bk_sb[:], bk_d.rearrange("(c p) -> p c", p=128))
            if has_bv:
                nc.sync.dma_start(bvA_sb[:], bvA_d[:])
                nc.sync.dma_start(bvB_sb[:], bvB_d[:])
            if not fast_mask:
                nc.sync.dma_start(mb_sb[:], mb_d.rearrange("(c p) -> p c", p=128))

            # persistent per-core projections
            q_sb = qkv.tile([128, PAIRS, L], F32R)      # q^T: [dim-in-pair, pair, token]
            k_sb = qkv.tile([128, PAIRS, L], F32R)
            vaug_sb = qkv.tile([128, L // 128, VAUG], EDT)  # [token-in-chunk, chunk, headcol]
            # constant ones columns for the deferred-softmax sum rows
            for h in range(8):
                nc.vector.memset(vaug_sb[:, :, h * VSTRIDE + 64:h * VSTRIDE + 65], 1.0)
            # loop-invariant weights
            wq_sb = qkv.tile([128, HCH, GDIM], F32R)
            wk_sb = qkv.tile([128, HCH, GDIM], F32R)
            wv_sb = qkv.tile([128, HCH, GDIM], F32R)
            nc.sync.dma_start(wq_sb[:], wq_d.rearrange("(c p) m -> p c m", p=128))
            nc.sync.dma_start(wk_sb[:], wk_d.rearrange("(c p) m -> p c m", p=128))
            nc.sync.dma_start(wv_sb[:], wv_d.rearrange("(c p) m -> p c m", p=128))

            def _emit_body():
                # ---------------- phase 1: Q/K projections ----------------
                # V projections are deferred into early phase 2 (below) where
                # they act as dependency-light PE filler that keeps the array
                # dense (and the clock un-throttled) through the phase
                # boundary and attention-pipeline ramp-up.
                xT_all = x_d.rearrange("(c p) t -> p c t", p=128)
                with (
                    tc.tile_pool(name="xt", bufs=2) as xtp,
                    tc.tile_pool(name="projps", bufs=2, space="PSUM") as projps,
                ):
                    for tci in range(TCH):
                        t0 = tci * 512
                        xt = xtp.tile([128, HCH, 512], F32R, tag="xt")
                        nc.sync.dma_start(xt[:], xT_all[:, :, t0:t0 + 512])
                        # q^T / k^T for each pair (dc), this token chunk
                        for dc in range(PAIRS):
                            qp = projps.tile([128, 512], F32, tag="qps")
                            kp = projps.tile([128, 512], F32, tag="kps")
                            for hc in range(HCH):
                                nc.tensor.matmul(
                                    qp[:], wq_sb[:, hc, dc * 128:(dc + 1) * 128],
                                    xt[:, hc, :],
                                    start=(hc == 0), stop=(hc == HCH - 1),
                                )
                            for hc in range(HCH):
                                nc.tensor.matmul(
                                    kp[:], wk_sb[:, hc, dc * 128:(dc + 1) * 128],
                                    xt[:, hc, :],
                                    start=(hc == 0), stop=(hc == HCH - 1),
                                )
                            if has_bqk:
                                nc.vector.tensor_scalar_add(
                                    q_sb[:, dc, t0:t0 + 512], qp[:], bq_sb[:, dc:dc + 1]
                                )
                                nc.vector.tensor_scalar_add(
                                    k_sb[:, dc, t0:t0 + 512], kp[:], bk_sb[:, dc:dc + 1]
                                )
                            else:
                                nc.vector.tensor_copy(q_sb[:, dc, t0:t0 + 512], qp[:])
                                nc.vector.tensor_copy(k_sb[:, dc, t0:t0 + 512], kp[:])

                # ---------------- phase 2: attention ----------------
                # Software-pipelined emission: scores are emitted two units
                # ahead of the exp/PV that consume them, so the tensor queue
                # never head-blocks on the ACT engine and the PE stays dense.
                with (
                    tc.tile_pool(name="epool", bufs=4) as epool,
                    tc.tile_pool(name="obuf", bufs=2) as obuf,
                    tc.tile_pool(name="xv", bufs=2) as xvp,
                    tc.tile_pool(name="scps", bufs=2, space="PSUM") as scps,
                    tc.tile_pool(name="ops", bufs=2, space="PSUM") as opsp,
                ):
                    def emit_vchains(cpair):
                        # V projection for key chunks 2*cpair, 2*cpair+1:
                        # woven into early attention as PE filler. PSUM comes
                        # from the scores ring (short-lived).
                        c0 = cpair * 2
                        xv = xvp.tile([128, HCH, 256], F32R, tag="xv")
                        nc.sync.dma_start(
                            xv[:], xT_all[:, :, c0 * 128:(c0 + 2) * 128]
                        )
                        for ci in range(2):
                            vp = scps.tile([128, 512], F32, tag="s")
                            for hc in range(HCH):
                                nc.tensor.matmul(
                                    vp[:], xv[:, hc, ci * 128:(ci + 1) * 128],
                                    wv_sb[:, hc, :],
                                    start=(hc == 0), stop=(hc == HCH - 1),
                                )
                            nc.vector.tensor_copy(
                                vaug_sb[:, c0 + ci, :].rearrange(
                                    "p (h s) -> p h s", h=8
                                )[:, :, 0:64],
                                vp[:].rearrange("p (h d) -> p h d", h=8),
                            )
                    KSTEPS = 8 if fast_mask else 16
                    NKC = 2 if fast_mask else 1    # key chunks per unit
                    SFREE = 512 * NKC
                    units = [
                        (p, qc, ks)
                        for p in range(PAIRS)
                        for qc in range(TCH)
                        for ks in range(KSTEPS)
                    ]
                    state = {}

                    def emit_scores(u):
                        p, qc, ks = u
                        q0 = qc * 512
                        sA = scps.tile([128, SFREE], F32, tag="s")
                        sB = scps.tile([128, SFREE], F32, tag="s")
                        for j in range(NKC):
                            kc = ks * NKC + j
                            # adjacent emission: rows 0-63 / 64-127 run
                            # concurrently in different row groups
                            nc.tensor.matmul(
                                sA[:, j * 512:(j + 1) * 512],
                                k_sb[0:64, p, kc * 128:(kc + 1) * 128],
                                q_sb[0:64, p, q0:q0 + 512],
                                start=True, stop=True,
                            )
                            nc.tensor.matmul(
                                sB[:, j * 512:(j + 1) * 512],
                                k_sb[64:128, p, kc * 128:(kc + 1) * 128],
                                q_sb[64:128, p, q0:q0 + 512],
                                start=True, stop=True,
                            )
                        state[u] = (sA, sB)

                    def emit_exp_pv(u, oA, oB):
                        p, qc, ks = u
                        hA, hB = 2 * p, 2 * p + 1
                        cA, cB = hA * VSTRIDE, hB * VSTRIDE
                        sA, sB = state.pop(u)
                        if ks in DVE_KCP and fast_mask:
                            eiA = epool.tile([128, SFREE], I16, tag="e")
                            eiB = epool.tile([128, SFREE], I16, tag="e")
                            nc.vector.tensor_scalar(
                                eiA[:], sA[:], EXP_A2, EXP_B2,
                                op0=ALU.mult, op1=ALU.add,
                            )
                            nc.vector.tensor_scalar(
                                eiB[:], sB[:], EXP_A2, EXP_B2,
                                op0=ALU.mult, op1=ALU.add,
                            )
                            eA = eiA.bitcast(F16)
                            eB = eiB.bitcast(F16)
                        else:
                            eA = epool.tile([128, SFREE], EDT, tag="e")
                            eB = epool.tile([128, SFREE], EDT, tag="e")
                            if fast_mask:
                                nc.scalar.activation(eA[:], sA[:], AF.Exp, scale=0.125)
                                nc.scalar.activation(eB[:], sB[:], AF.Exp, scale=0.125)
                            else:
                                kc = ks
                                nc.scalar.activation(
                                    eA[:], sA[:], AF.Exp,
                                    bias=mb_sb[:, kc:kc + 1], scale=0.125,
                                )
                                nc.scalar.activation(
                                    eB[:], sB[:], AF.Exp,
                                    bias=mb_sb[:, kc:kc + 1], scale=0.125,
                                )
                        for j in range(NKC):
                            kc = ks * NKC + j
                            first = kc == 0
                            last = kc == L // 128 - 1
                            nc.tensor.matmul(
                                oA[:], vaug_sb[:, kc, cA:cA + 65],
                                eA[:, j * 512:(j + 1) * 512],
                                start=first, stop=last,
                            )
                            nc.tensor.matmul(
                                oB[:], vaug_sb[:, kc, cB:cB + 65],
                                eB[:, j * 512:(j + 1) * 512],
                                start=first, stop=last,
                            )

                    def emit_norm(p, qc, oA, oB):
                        hA, hB = 2 * p, 2 * p + 1
                        q0 = qc * 512
                        denA = obuf.tile([1, 512], F32, tag="denA")
                        denB = obuf.tile([1, 512], F32, tag="denB")
                        nc.vector.tensor_copy(denA[:], oA[64:65, :])
                        nc.vector.tensor_copy(denB[:], oB[64:65, :])
                        recA = obuf.tile([1, 512], F32, tag="recA")
                        recB = obuf.tile([1, 512], F32, tag="recB")
                        nc.vector.reciprocal_approx_fast(recA[:], denA[:])
                        nc.vector.reciprocal_approx_fast(recB[:], denB[:])
                        for idx, (o_ps, rec, h, bv_sb) in enumerate((
                            (oA, recA, hA, bvA_sb),
                            (oB, recB, hB, bvB_sb),
                        )):
                            bc_sb = obuf.tile([64, 512], F32, tag=f"bcsb{idx}")
                            nc.gpsimd.partition_broadcast(bc_sb[:], rec[:])
                            o_sb = obuf.tile([64, 512], F32, tag=f"osb{idx}")
                            nc.vector.tensor_tensor(
                                out=o_sb[:], in0=o_ps[0:64, :], in1=bc_sb[:],
                                op=ALU.mult,
                            )
                            if has_bv:
                                nc.vector.tensor_scalar_add(
                                    o_sb[:], o_sb[:], bv_sb[:, p:p + 1]
                                )
                            d0 = p * 128 + (h % 2) * 64
                            nc.sync.dma_start(
                                out_d[d0:d0 + 64, q0:q0 + 512], o_sb[:]
                            )

                    def emit_scores_at(j):
                        # weave the deferred V projections ahead of the first
                        # few units' scores
                        if fast_mask:
                            if j < 8:
                                emit_vchains(j)
                        else:
                            if j < 16 and j % 2 == 0:
                                emit_vchains(j // 2)
                        emit_scores(units[j])

                    emit_scores_at(0)
                    emit_scores_at(1)
                    o_cur = None
                    for i, u in enumerate(units):
                        p, qc, ks = u
                        if ks == 0:
                            oA = opsp.tile([65, 512], F32, tag="oA")
                            oB = opsp.tile([65, 512], F32, tag="oB")
                            o_cur = (oA, oB)
                        # PV first: it unblocks on exp(i) which completes
                        # before scores(i+2)'s ring slot frees, so the PE
                        # never head-blocks on a later dependency.
                        emit_exp_pv(u, *o_cur)
                        if i + 2 < len(units):
                            emit_scores_at(i + 2)
                        if ks == KSTEPS - 1:
                            emit_norm(p, qc, *o_cur)

            loop_cm = (
                tc.For_i(0, repeat, 1) if repeat > 1 else contextlib.nullcontext()
            )
            with loop_cm:
                _emit_body()

    nc.finalize()
    return nc


def _get_nc(fast_mask: bool, has_bqk: bool, has_bv: bool):
    key = (fast_mask, has_bqk, has_bv)
    if key not in _NC_CACHE:
        _NC_CACHE[key] = _build(*key)
    return _NC_CACHE[key]


def _prep_in_maps(x, masked_attention, Wq, bq, Wk, bk, Wv, bv):
    x = np.asarray(x, np.float32)
    mask = np.asarray(masked_attention, np.float32)
    Wq = np.asarray(Wq, np.float32)
    Wk = np.asarray(Wk, np.float32)
    Wv = np.asarray(Wv, np.float32)
    bq = np.asarray(bq, np.float32)
    bk = np.asarray(bk, np.float32)
    bv = np.asarray(bv, np.float32)

    ones = np.ones((1, 128), np.float32)
    maskb = (mask - 1.0) * 10000.0

    per_g = []
    for g in range(2):
        sl = slice(g * GDIM, (g + 1) * GDIM)
        wqT = np.ascontiguousarray(Wq[sl, :].T)
        wkT = np.ascontiguousarray(Wk[sl, :].T)
        wvT = np.ascontiguousarray(Wv[sl, :].T)
        bq_g = bq[sl].copy()
        bk_g = bk[sl].copy()
        bv_g = bv[sl].reshape(8, 64)
        bvA = np.ascontiguousarray(bv_g[0::2].T)  # [64, PAIRS]
        bvB = np.ascontiguousarray(bv_g[1::2].T)
        per_g.append((wqT, wkT, wvT, bq_g, bk_g, bvA, bvB))

    xT = [np.ascontiguousarray(x[b].T) for b in range(B)]

    in_maps = []
    for c in range(N_CORES):
        g, b = divmod(c, B)
        wqT, wkT, wvT, bq_g, bk_g, bvA, bvB = per_g[g]
        in_maps.append({
            "xT": xT[b],
            "wqT": wqT, "wkT": wkT, "wvT": wvT,
            "bq": bq_g, "bk": bk_g, "bvA": bvA, "bvB": bvB,
            "maskb": np.ascontiguousarray(maskb[b]),
            "ones": ones,
        })

    fast_mask = bool(np.all(mask == 1.0))
    has_bqk = bool(np.any(bq) or np.any(bk))
    has_bv = bool(np.any(bv))
    return in_maps, fast_mask, has_bqk, has_bv


def _gather(results):
    out = np.empty((B, L, HIDDEN), np.float32)
    for c in range(N_CORES):
        g, b = divmod(c, B)
        out[b, :, g * GDIM:(g + 1) * GDIM] = results[c]["out"].T
    return out


def kernel(x, masked_attention, Wq, bq, Wk, bk, Wv, bv):
    in_maps, fast_mask, has_bqk, has_bv = _prep_in_maps(
        x, masked_attention, Wq, bq, Wk, bk, Wv, bv
    )
    nc = _get_nc(fast_mask, has_bqk, has_bv)
    res = run_bass_kernel_spmd(nc, in_maps, core_ids=list(range(N_CORES)))
    return _gather(res.results)


# revision 31
# speedup vs baseline: 1.2248x; 1.0345x over previous
"""BERT self-attention (B=4, L=2048, H=1024, 16 heads) on 8 trn2 NeuronCores.

Sharding: core c = (g, b) with b = batch index (4) and g = head-half (2).
Each core computes Q/K/V projections for its 8 heads over its batch, then
full attention for those heads, producing out[b, :, g*512:(g+1)*512].

On-core layout is "transposed": x arrives pre-transposed from the host
(x^T[hidden, token]), projections produce q^T / k^T with head-dim on
partitions, scores are computed transposed (s^T[key, query]) so softmax'd
probabilities land directly in the layout the P@V matmul needs (keys on the
contraction/partition dim) — no O(L^2) transposes. Softmax skips the max
subtraction (scores ~ N(0,1): exp is safe in fp32) and the normalization is
deferred: V is augmented with a constant ones column (memset once) so each
P@V matmul also yields the exp-sum row, and the division happens once on the
[64, 512] output tile via a fast approximate reciprocal.

Scores matmuls for the two heads of a pair contract over disjoint partition
halves (rows 0-63 / 64-127) and are emitted adjacently so the PE runs them
concurrently in different row groups. A fraction of the exp work can be
offloaded from the Scalar engine to the Vector engine via a one-instruction
Schraudolph-style fast exp that emits float16 bit patterns directly
(DVE_KCP below), balancing the two engines.
"""

import contextlib
import os
import sys

for _p in ("/opt/trn_rl_repo",):
    if os.path.isdir(_p) and _p not in sys.path:
        sys.path.insert(0, _p)

import numpy as np

import concourse.bass as bass
import concourse.tile as tile
from concourse import bacc, mybir
from concourse.bass_utils import run_bass_kernel_spmd

F32 = mybir.dt.float32
F32R = mybir.dt.float32r
F16 = mybir.dt.float16
I16 = mybir.dt.int16
AF = mybir.ActivationFunctionType
ALU = mybir.AluOpType

B, L, HIDDEN = 4, 2048, 1024
NH, D = 16, 64
N_CORES = 8
GDIM = 512            # output dims per core (8 heads x 64)
PAIRS = 4             # head pairs per core (2 heads share a 128-partition group)
TCH = 4               # token chunks of 512
HCH = 8               # hidden chunks of 128
VSTRIDE = 66          # per-head stride in vaug: 64 dims + 1 ones col + 1 pad
VAUG = 8 * VSTRIDE    # 528 cols per 128-token chunk

# kcp indices (of 8) whose exp runs on the Vector engine (fast-exp) instead
# of the Scalar engine. () disables the offload.
DVE_KCP = ()
# fast-exp constants: f16 bits of exp(s/8) ~= int16(A2*s + B2)
EXP_A2 = 0.125 * 1.4426950408889634 * 1024.0
EXP_B2 = 15.0 * 1024.0 - 44.0   # magic offset tuned for min rel error

_NC_CACHE = {}


def _build(fast_mask: bool, has_bqk: bool, has_bv: bool, repeat: int = 1):
    EDT = F16
    nc = bacc.Bacc("TRN2", target_bir_lowering=False, debug=False)
    x_d = nc.dram_tensor("xT", [HIDDEN, L], F32R, kind="ExternalInput")
    wq_d = nc.dram_tensor("wqT", [HIDDEN, GDIM], F32R, kind="ExternalInput")
    wk_d = nc.dram_tensor("wkT", [HIDDEN, GDIM], F32R, kind="ExternalInput")
    wv_d = nc.dram_tensor("wvT", [HIDDEN, GDIM], F32R, kind="ExternalInput")
    bq_d = nc.dram_tensor("bq", [GDIM], F32, kind="ExternalInput")
    bk_d = nc.dram_tensor("bk", [GDIM], F32, kind="ExternalInput")
    bvA_d = nc.dram_tensor("bvA", [64, PAIRS], F32, kind="ExternalInput")
    bvB_d = nc.dram_tensor("bvB", [64, PAIRS], F32, kind="ExternalInput")
    mb_d = nc.dram_tensor("maskb", [L], F32, kind="ExternalInput")
    ones_d = nc.dram_tensor("ones", [1, 128], F32R, kind="ExternalInput")
    out_d = nc.dram_tensor("out", [GDIM, L], F32, kind="ExternalOutput")

    with nc.allow_low_precision(reason="fp32r attention"), tile.TileContext(nc) as tc:
        with (
            tc.tile_pool(name="consts", bufs=1) as consts,
            tc.tile_pool(name="qkv", bufs=1) as qkv,
        ):
            bq_sb = consts.tile([128, PAIRS], F32)
            bk_sb = consts.tile([128, PAIRS], F32)
            bvA_sb = consts.tile([64, PAIRS], F32)
            bvB_sb = consts.tile([64, PAIRS], F32)
            mb_sb = consts.tile([128, 16], F32)
            if has_bqk:
                nc.sync.dma_start(bq_sb[:], bq_d.rearrange("(c p) -> p c", p=128))
                nc.sync.dma_start(bk_sb[:], bk_d.rearrange("(c p) -> p c", p=128))
            if has_bv:
                nc.sync.dma_start(bvA_sb[:], bvA_d[:])
                nc.sync.dma_start(bvB_sb[:], bvB_d[:])
            if not fast_mask:
                nc.sync.dma_start(mb_sb[:], mb_d.rearrange("(c p) -> p c", p=128))

            # persistent per-core projections
            q_sb = qkv.tile([128, PAIRS, L], F32R)      # q^T: [dim-in-pair, pair, token]
            k_sb = qkv.tile([128, PAIRS, L], F32R)
            vaug_sb = qkv.tile([128, L // 128, VAUG], EDT)  # [token-in-chunk, chunk, headcol]
            # constant ones columns for the deferred-softmax sum rows
            for h in range(8):
                nc.vector.memset(vaug_sb[:, :, h * VSTRIDE + 64:h * VSTRIDE + 65], 1.0)
            # loop-invariant weights
            wq_sb = qkv.tile([128, HCH, GDIM], F32R)
            wk_sb = qkv.tile([128, HCH, GDIM], F32R)
            wv_sb = qkv.tile([128, HCH, GDIM], F32R)
            nc.sync.dma_start(wq_sb[:], wq_d.rearrange("(c p) m -> p c m", p=128))
            nc.sync.dma_start(wk_sb[:], wk_d.rearrange("(c p) m -> p c m", p=128))
            nc.sync.dma_start(wv_sb[:], wv_d.rearrange("(c p) m -> p c m", p=128))

            def _emit_body():
                # ---------------- phase 1: Q/K projections ----------------
                # V projections are deferred into early phase 2 (below) where
                # they act as dependency-light PE filler that keeps the array
                # dense (and the clock un-throttled) through the phase
                # boundary and attention-pipeline ramp-up.
                xT_all = x_d.rearrange("(c p) t -> p c t", p=128)
                with (
                    tc.tile_pool(name="xt", bufs=2) as xtp,
                    tc.tile_pool(name="projps", bufs=2, space="PSUM") as projps,
                ):
                    for tci in range(TCH):
                        t0 = tci * 512
                        xt = xtp.tile([128, HCH, 512], F32R, tag="xt")
                        nc.sync.dma_start(xt[:], xT_all[:, :, t0:t0 + 512])
                        # q^T / k^T for each pair (dc), this token chunk
                        for dc in range(PAIRS):
                            qp = projps.tile([128, 512], F32, tag="qps")
                            kp = projps.tile([128, 512], F32, tag="kps")
                            for hc in range(HCH):
                                nc.tensor.matmul(
                                    qp[:], wq_sb[:, hc, dc * 128:(dc + 1) * 128],
                                    xt[:, hc, :],
                                    start=(hc == 0), stop=(hc == HCH - 1),
                                )
                            for hc in range(HCH):
                                nc.tensor.matmul(
                                    kp[:], wk_sb[:, hc, dc * 128:(dc + 1) * 128],
                                    xt[:, hc, :],
                                    start=(hc == 0), stop=(hc == HCH - 1),
                                )
                            if has_bqk:
                                nc.vector.tensor_scalar_add(
                                    q_sb[:, dc, t0:t0 + 512], qp[:], bq_sb[:, dc:dc + 1]
                                )
                                nc.vector.tensor_scalar_add(
                                    k_sb[:, dc, t0:t0 + 512], kp[:], bk_sb[:, dc:dc + 1]
                                )
                            else:
                                nc.vector.tensor_copy(q_sb[:, dc, t0:t0 + 512], qp[:])
                                nc.vector.tensor_copy(k_sb[:, dc, t0:t0 + 512], kp[:])

                # ---------------- phase 2: attention ----------------
                # Software-pipelined emission: scores are emitted two units
                # ahead of the exp/PV that consume them, so the tensor queue
                # never head-blocks on the ACT engine and the PE stays dense.
                with (
                    tc.tile_pool(name="epool", bufs=4) as epool,
                    tc.tile_pool(name="obuf", bufs=2) as obuf,
                    tc.tile_pool(name="xv", bufs=2) as xvp,
                    tc.tile_pool(name="scps", bufs=2, space="PSUM") as scps,
                    tc.tile_pool(name="ops", bufs=2, space="PSUM") as opsp,
                ):
                    def emit_vchains(cpair):
                        # V projection for key chunks 2*cpair, 2*cpair+1:
                        # woven into early attention as PE filler. PSUM comes
                        # from the scores ring (short-lived).
                        c0 = cpair * 2
                        xv = xvp.tile([128, HCH, 256], F32R, tag="xv")
                        nc.sync.dma_start(
                            xv[:], xT_all[:, :, c0 * 128:(c0 + 2) * 128]
                        )
                        for ci in range(2):
                            vp = scps.tile([128, 512], F32, tag="s")
                            for hc in range(HCH):
                                nc.tensor.matmul(
                                    vp[:], xv[:, hc, ci * 128:(ci + 1) * 128],
                                    wv_sb[:, hc, :],
                                    start=(hc == 0), stop=(hc == HCH - 1),
                                )
                            nc.vector.tensor_copy(
                                vaug_sb[:, c0 + ci, :].rearrange(
                                    "p (h s) -> p h s", h=8
                                )[:, :, 0:64],
                                vp[:].rearrange("p (h d) -> p h d", h=8),
                            )
                    KSTEPS = 16
                    units = [
                        (p, qc, kc)
                        for p in range(PAIRS)
                        for qc in range(TCH)
                        for kc in range(KSTEPS)
                    ]
                    state = {}

                    def emit_scores(u):
                        p, qc, kc = u
                        q0 = qc * 512
                        # both heads share one tile, freed by a single exp:
                        # their next-unit scores become ready simultaneously,
                        # so the scheduler keeps the pair adjacent and rows
                        # 0-63 / 64-127 run concurrently in the PE array.
                        s = scps.tile([128, 1024], F32, tag="s")
                        nc.tensor.matmul(
                            s[:, 0:512],
                            k_sb[0:64, p, kc * 128:(kc + 1) * 128],
                            q_sb[0:64, p, q0:q0 + 512],
                            start=True, stop=True,
                        )
                        nc.tensor.matmul(
                            s[:, 512:1024],
                            k_sb[64:128, p, kc * 128:(kc + 1) * 128],
                            q_sb[64:128, p, q0:q0 + 512],
                            start=True, stop=True,
                        )
                        state[u] = s

                    def emit_exp_pv(u, oA, oB):
                        p, qc, kc = u
                        hA, hB = 2 * p, 2 * p + 1
                        cA, cB = hA * VSTRIDE, hB * VSTRIDE
                        s = state.pop(u)
                        if kc in DVE_KCP and fast_mask:
                            ei = epool.tile([128, 1024], I16, tag="e")
                            nc.vector.tensor_scalar(
                                ei[:], s[:], EXP_A2, EXP_B2,
                                op0=ALU.mult, op1=ALU.add,
                            )
                            e = ei.bitcast(F16)
                        else:
                            e = epool.tile([128, 1024], EDT, tag="e")
                            if fast_mask:
                                nc.scalar.activation(e[:], s[:], AF.Exp, scale=0.125)
                            else:
                                # mask bias is per key (partition) — identical
                                # for both heads' column halves
                                nc.scalar.activation(
                                    e[:], s[:], AF.Exp,
                                    bias=mb_sb[:, kc:kc + 1], scale=0.125,
                                )
                        first = kc == 0
                        last = kc == KSTEPS - 1
                        nc.tensor.matmul(
                            oA[:], vaug_sb[:, kc, cA:cA + 65],
                            e[:, 0:512],
                            start=first, stop=last,
                        )
                        nc.tensor.matmul(
                            oB[:], vaug_sb[:, kc, cB:cB + 65],
                            e[:, 512:1024],
                            start=first, stop=last,
                        )

                    def emit_norm(p, qc, oA, oB):
                        hA, hB = 2 * p, 2 * p + 1
                        q0 = qc * 512
                        denA = obuf.tile([1, 512], F32, tag="denA")
                        denB = obuf.tile([1, 512], F32, tag="denB")
                        nc.vector.tensor_copy(denA[:], oA[64:65, :])
                        nc.vector.tensor_copy(denB[:], oB[64:65, :])
                        recA = obuf.tile([1, 512], F32, tag="recA")
                        recB = obuf.tile([1, 512], F32, tag="recB")
                        nc.vector.reciprocal_approx_fast(recA[:], denA[:])
                        nc.vector.reciprocal_approx_fast(recB[:], denB[:])
                        for idx, (o_ps, rec, h, bv_sb) in enumerate((
                            (oA, recA, hA, bvA_sb),
                            (oB, recB, hB, bvB_sb),
                        )):
                            bc_sb = obuf.tile([64, 512], F32, tag=f"bcsb{idx}")
                            nc.gpsimd.partition_broadcast(bc_sb[:], rec[:])
                            o_sb = obuf.tile([64, 512], F32, tag=f"osb{idx}")
                            nc.vector.tensor_tensor(
                                out=o_sb[:], in0=o_ps[0:64, :], in1=bc_sb[:],
                                op=ALU.mult,
                            )
                            if has_bv:
                                nc.vector.tensor_scalar_add(
                                    o_sb[:], o_sb[:], bv_sb[:, p:p + 1]
                                )
                            d0 = p * 128 + (h % 2) * 64
                            nc.sync.dma_start(
                                out_d[d0:d0 + 64, q0:q0 + 512], o_sb[:]
                            )

                    def emit_scores_at(j):
                        # weave the deferred V projections ahead of the first
                        # few units' scores (chunk pair j//2 covers the keys
                        # units j and j+1 consume)
                        if j < 16 and j % 2 == 0:
                            emit_vchains(j // 2)
                        emit_scores(units[j])

                    emit_scores_at(0)
                    emit_scores_at(1)
                    o_cur = None
                    for i, u in enumerate(units):
                        p, qc, ks = u
                        if ks == 0:
                            oA = opsp.tile([65, 512], F32, tag="oA")
                            oB = opsp.tile([65, 512], F32, tag="oB")
                            o_cur = (oA, oB)
                        # PV first: it unblocks on exp(i) which completes
                        # before scores(i+2)'s ring slot frees, so the PE
                        # never head-blocks on a later dependency.
                        emit_exp_pv(u, *o_cur)
                        if i + 2 < len(units):
                            emit_scores_at(i + 2)
                        if ks == KSTEPS - 1:
                            emit_norm(p, qc, *o_cur)

            loop_cm = (
                tc.For_i(0, repeat, 1) if repeat > 1 else contextlib.nullcontext()
            )
            with loop_cm:
                _emit_body()

    nc.finalize()
    return nc


def _get_nc(fast_mask: bool, has_bqk: bool, has_bv: bool):
    key = (fast_mask, has_bqk, has_bv)
    if key not in _NC_CACHE:
        _NC_CACHE[key] = _build(*key)
    return _NC_CACHE[key]


def _prep_in_maps(x, masked_attention, Wq, bq, Wk, bk, Wv, bv):
    x = np.asarray(x, np.float32)
    mask = np.asarray(masked_attention, np.float32)
    Wq = np.asarray(Wq, np.float32)
    Wk = np.asarray(Wk, np.float32)
    Wv = np.asarray(Wv, np.float32)
    bq = np.asarray(bq, np.float32)
    bk = np.asarray(bk, np.float32)
    bv = np.asarray(bv, np.float32)

    ones = np.ones((1, 128), np.float32)
    maskb = (mask - 1.0) * 10000.0

    per_g = []
    for g in range(2):
        sl = slice(g * GDIM, (g + 1) * GDIM)
        wqT = np.ascontiguousarray(Wq[sl, :].T)
        wkT = np.ascontiguousarray(Wk[sl, :].T)
        wvT = np.ascontiguousarray(Wv[sl, :].T)
        bq_g = bq[sl].copy()
        bk_g = bk[sl].copy()
        bv_g = bv[sl].reshape(8, 64)
        bvA = np.ascontiguousarray(bv_g[0::2].T)  # [64, PAIRS]
        bvB = np.ascontiguousarray(bv_g[1::2].T)
        per_g.append((wqT, wkT, wvT, bq_g, bk_g, bvA, bvB))

    xT = [np.ascontiguousarray(x[b].T) for b in range(B)]

    in_maps = []
    for c in range(N_CORES):
        g, b = divmod(c, B)
        wqT, wkT, wvT, bq_g, bk_g, bvA, bvB = per_g[g]
        in_maps.append({
            "xT": xT[b],
            "wqT": wqT, "wkT": wkT, "wvT": wvT,
            "bq": bq_g, "bk": bk_g, "bvA": bvA, "bvB": bvB,
            "maskb": np.ascontiguousarray(maskb[b]),
            "ones": ones,
        })

    fast_mask = bool(np.all(mask == 1.0))
    has_bqk = bool(np.any(bq) or np.any(bk))
    has_bv = bool(np.any(bv))
    return in_maps, fast_mask, has_bqk, has_bv


def _gather(results):
    out = np.empty((B, L, HIDDEN), np.float32)
    for c in range(N_CORES):
        g, b = divmod(c, B)
        out[b, :, g * GDIM:(g + 1) * GDIM] = results[c]["out"].T
    return out


def kernel(x, masked_attention, Wq, bq, Wk, bk, Wv, bv):
    in_maps, fast_mask, has_bqk, has_bv = _prep_in_maps(
        x, masked_attention, Wq, bq, Wk, bk, Wv, bv
    )
    nc = _get_nc(fast_mask, has_bqk, has_bv)
    res = run_bass_kernel_spmd(nc, in_maps, core_ids=list(range(N_CORES)))
    return _gather(res.results)


# revision 33
# speedup vs baseline: 1.2430x; 1.0148x over previous
"""BERT self-attention (B=4, L=2048, H=1024, 16 heads) on 8 trn2 NeuronCores.

Sharding: core c = (g, b) with b = batch index (4) and g = head-half (2).
Each core computes Q/K/V projections for its 8 heads over its batch, then
full attention for those heads, producing out[b, :, g*512:(g+1)*512].

On-core layout is "transposed": x arrives pre-transposed from the host
(x^T[hidden, token]), projections produce q^T / k^T with head-dim on
partitions, scores are computed transposed (s^T[key, query]) so softmax'd
probabilities land directly in the layout the P@V matmul needs (keys on the
contraction/partition dim) — no O(L^2) transposes. Softmax skips the max
subtraction (scores ~ N(0,1): exp is safe in fp32) and the normalization is
deferred: V is augmented with a constant ones column (memset once) so each
P@V matmul also yields the exp-sum row, and the division happens once on the
[64, 512] output tile via a fast approximate reciprocal.

Scores matmuls for the two heads of a pair contract over disjoint partition
halves (rows 0-63 / 64-127) and are emitted adjacently so the PE runs them
concurrently in different row groups. A fraction of the exp work can be
offloaded from the Scalar engine to the Vector engine via a one-instruction
Schraudolph-style fast exp that emits float16 bit patterns directly
(DVE_KCP below), balancing the two engines.
"""

import contextlib
import os
import sys

for _p in ("/opt/trn_rl_repo",):
    if os.path.isdir(_p) and _p not in sys.path:
        sys.path.insert(0, _p)

import numpy as np

import concourse.bass as bass
import concourse.tile as tile
from concourse import bacc, mybir
from concourse.bass_utils import run_bass_kernel_spmd

F32 = mybir.dt.float32
F32R = mybir.dt.float32r
F16 = mybir.dt.float16
I16 = mybir.dt.int16
AF = mybir.ActivationFunctionType
ALU = mybir.AluOpType

B, L, HIDDEN = 4, 2048, 1024
NH, D = 16, 64
N_CORES = 8
GDIM = 512            # output dims per core (8 heads x 64)
PAIRS = 4             # head pairs per core (2 heads share a 128-partition group)
TCH = 4               # token chunks of 512
HCH = 8               # hidden chunks of 128
VSTRIDE = 66          # per-head stride in vaug: 64 dims + 1 ones col + 1 pad
VAUG = 8 * VSTRIDE    # 528 cols per 128-token chunk

# kcp indices (of 8) whose exp runs on the Vector engine (fast-exp) instead
# of the Scalar engine. () disables the offload.
DVE_KCP = ()
# fast-exp constants: f16 bits of exp(s/8) ~= int16(A2*s + B2)
EXP_A2 = 0.125 * 1.4426950408889634 * 1024.0
EXP_B2 = 15.0 * 1024.0 - 44.0   # magic offset tuned for min rel error

_NC_CACHE = {}


def _build(fast_mask: bool, has_bqk: bool, has_bv: bool, repeat: int = 1):
    EDT = F16
    nc = bacc.Bacc("TRN2", target_bir_lowering=False, debug=False)
    x_d = nc.dram_tensor("xT", [HIDDEN, L], F32R, kind="ExternalInput")
    wq_d = nc.dram_tensor("wqT", [HIDDEN, GDIM], F32R, kind="ExternalInput")
    wk_d = nc.dram_tensor("wkT", [HIDDEN, GDIM], F32R, kind="ExternalInput")
    wv_d = nc.dram_tensor("wvT", [HIDDEN, GDIM], F32R, kind="ExternalInput")
    bq_d = nc.dram_tensor("bq", [GDIM], F32, kind="ExternalInput")
    bk_d = nc.dram_tensor("bk", [GDIM], F32, kind="ExternalInput")
    bvA_d = nc.dram_tensor("bvA", [64, PAIRS], F32, kind="ExternalInput")
    bvB_d = nc.dram_tensor("bvB", [64, PAIRS], F32, kind="ExternalInput")
    mb_d = nc.dram_tensor("maskb", [L], F32, kind="ExternalInput")
    ones_d = nc.dram_tensor("ones", [1, 128], F32R, kind="ExternalInput")
    out_d = nc.dram_tensor("out", [GDIM, L], F32, kind="ExternalOutput")

    with nc.allow_low_precision(reason="fp32r attention"), tile.TileContext(nc) as tc:
        with (
            tc.tile_pool(name="consts", bufs=1) as consts,
            tc.tile_pool(name="qkv", bufs=1) as qkv,
        ):
            bq_sb = consts.tile([128, PAIRS], F32)
            bk_sb = consts.tile([128, PAIRS], F32)
            bvA_sb = consts.tile([64, PAIRS], F32)
            bvB_sb = consts.tile([64, PAIRS], F32)
            mb_sb = consts.tile([128, 16], F32)
            if has_bqk:
                nc.sync.dma_start(bq_sb[:], bq_d.rearrange("(c p) -> p c", p=128))
                nc.sync.dma_start(bk_sb[:], bk_d.rearrange("(c p) -> p c", p=128))
            if has_bv:
                nc.sync.dma_start(bvA_sb[:], bvA_d[:])
                nc.sync.dma_start(bvB_sb[:], bvB_d[:])
            if not fast_mask:
                nc.sync.dma_start(mb_sb[:], mb_d.rearrange("(c p) -> p c", p=128))

            # persistent per-core projections
            q_sb = qkv.tile([128, PAIRS, L], F32R)      # q^T: [dim-in-pair, pair, token]
            k_sb = qkv.tile([128, PAIRS, L], F32R)
            vaug_sb = qkv.tile([128, L // 128, VAUG], EDT)  # [token-in-chunk, chunk, headcol]
            # constant ones columns for the deferred-softmax sum rows
            for h in range(8):
                nc.vector.memset(vaug_sb[:, :, h * VSTRIDE + 64:h * VSTRIDE + 65], 1.0)
            # loop-invariant weights
            wq_sb = qkv.tile([128, HCH, GDIM], F32R)
            wk_sb = qkv.tile([128, HCH, GDIM], F32R)
            wv_sb = qkv.tile([128, HCH, GDIM], F32R)
            nc.sync.dma_start(wq_sb[:], wq_d.rearrange("(c p) m -> p c m", p=128))
            nc.sync.dma_start(wk_sb[:], wk_d.rearrange("(c p) m -> p c m", p=128))

            def _emit_body():
                # ---------------- phase 1: Q/K projections ----------------
                # V projections are deferred into early phase 2 (below) where
                # they act as dependency-light PE filler that keeps the array
                # dense (and the clock un-throttled) through the phase
                # boundary and attention-pipeline ramp-up.
                xT_all = x_d.rearrange("(c p) t -> p c t", p=128)
                with (
                    tc.tile_pool(name="xt", bufs=2) as xtp,
                    tc.tile_pool(name="projps", bufs=2, space="PSUM") as projps,
                ):
                    for tci in range(TCH):
                        t0 = tci * 512
                        xt = xtp.tile([128, HCH, 512], F32R, tag="xt")
                        nc.sync.dma_start(xt[:], xT_all[:, :, t0:t0 + 512])
                        # q^T / k^T for each pair (dc), this token chunk
                        for dc in range(PAIRS):
                            qp = projps.tile([128, 512], F32, tag="qps")
                            kp = projps.tile([128, 512], F32, tag="kps")
                            for hc in range(HCH):
                                nc.tensor.matmul(
                                    qp[:], wq_sb[:, hc, dc * 128:(dc + 1) * 128],
                                    xt[:, hc, :],
                                    start=(hc == 0), stop=(hc == HCH - 1),
                                )
                            for hc in range(HCH):
                                nc.tensor.matmul(
                                    kp[:], wk_sb[:, hc, dc * 128:(dc + 1) * 128],
                                    xt[:, hc, :],
                                    start=(hc == 0), stop=(hc == HCH - 1),
                                )
                            if has_bqk:
                                nc.vector.tensor_scalar_add(
                                    q_sb[:, dc, t0:t0 + 512], qp[:], bq_sb[:, dc:dc + 1]
                                )
                                nc.vector.tensor_scalar_add(
                                    k_sb[:, dc, t0:t0 + 512], kp[:], bk_sb[:, dc:dc + 1]
                                )
                            else:
                                nc.vector.tensor_copy(q_sb[:, dc, t0:t0 + 512], qp[:])
                                nc.vector.tensor_copy(k_sb[:, dc, t0:t0 + 512], kp[:])

                # ---------------- phase 2: attention ----------------
                # wv is only consumed by the V-weave below; emitting its DMA
                # here keeps it from competing with the ramp-critical
                # wq/wk/xt transfers at kernel start.
                nc.sync.dma_start(wv_sb[:], wv_d.rearrange("(c p) m -> p c m", p=128))
                # Software-pipelined emission: scores are emitted two units
                # ahead of the exp/PV that consume them, so the tensor queue
                # never head-blocks on the ACT engine and the PE stays dense.
                with (
                    tc.tile_pool(name="epool", bufs=6) as epool,
                    tc.tile_pool(name="obuf", bufs=3) as obuf,
                    tc.tile_pool(name="xv", bufs=2) as xvp,
                    tc.tile_pool(name="scps", bufs=2, space="PSUM") as scps,
                    tc.tile_pool(name="ops", bufs=2, space="PSUM") as opsp,
                ):
                    def emit_vchains(cpair):
                        # V projection for key chunks 2*cpair, 2*cpair+1:
                        # woven into early attention as PE filler. PSUM comes
                        # from the scores ring (short-lived).
                        c0 = cpair * 2
                        xv = xvp.tile([128, HCH, 256], F32R, tag="xv")
                        nc.sync.dma_start(
                            xv[:], xT_all[:, :, c0 * 128:(c0 + 2) * 128]
                        )
                        for ci in range(2):
                            vp = scps.tile([128, 512], F32, tag="s")
                            for hc in range(HCH):
                                nc.tensor.matmul(
                                    vp[:], xv[:, hc, ci * 128:(ci + 1) * 128],
                                    wv_sb[:, hc, :],
                                    start=(hc == 0), stop=(hc == HCH - 1),
                                )
                            nc.vector.tensor_copy(
                                vaug_sb[:, c0 + ci, :].rearrange(
                                    "p (h s) -> p h s", h=8
                                )[:, :, 0:64],
                                vp[:].rearrange("p (h d) -> p h d", h=8),
                            )
                    KSTEPS = 16
                    units = [
                        (p, qc, kc)
                        for p in range(PAIRS)
                        for qc in range(TCH)
                        for kc in range(KSTEPS)
                    ]
                    state = {}

                    def emit_scores(u):
                        p, qc, kc = u
                        q0 = qc * 512
                        # both heads share one tile, freed by a single exp:
                        # their next-unit scores become ready simultaneously,
                        # so the scheduler keeps the pair adjacent and rows
                        # 0-63 / 64-127 run concurrently in the PE array.
                        s = scps.tile([128, 1024], F32, tag="s")
                        nc.tensor.matmul(
                            s[:, 0:512],
                            k_sb[0:64, p, kc * 128:(kc + 1) * 128],
                            q_sb[0:64, p, q0:q0 + 512],
                            start=True, stop=True,
                        )
                        nc.tensor.matmul(
                            s[:, 512:1024],
                            k_sb[64:128, p, kc * 128:(kc + 1) * 128],
                            q_sb[64:128, p, q0:q0 + 512],
                            start=True, stop=True,
                        )
                        state[u] = s

                    def emit_exp_pv(u, oA, oB):
                        p, qc, kc = u
                        hA, hB = 2 * p, 2 * p + 1
                        cA, cB = hA * VSTRIDE, hB * VSTRIDE
                        s = state.pop(u)
                        if kc in DVE_KCP and fast_mask:
                            ei = epool.tile([128, 1024], I16, tag="e")
                            nc.vector.tensor_scalar(
                                ei[:], s[:], EXP_A2, EXP_B2,
                                op0=ALU.mult, op1=ALU.add,
                            )
                            e = ei.bitcast(F16)
                        else:
                            e = epool.tile([128, 1024], EDT, tag="e")
                            if fast_mask:
                                nc.scalar.activation(e[:], s[:], AF.Exp, scale=0.125)
                            else:
                                # mask bias is per key (partition) — identical
                                # for both heads' column halves
                                nc.scalar.activation(
                                    e[:], s[:], AF.Exp,
                                    bias=mb_sb[:, kc:kc + 1], scale=0.125,
                                )
                        first = kc == 0
                        last = kc == KSTEPS - 1
                        nc.tensor.matmul(
                            oA[:], vaug_sb[:, kc, cA:cA + 65],
                            e[:, 0:512],
                            start=first, stop=last,
                        )
                        nc.tensor.matmul(
                            oB[:], vaug_sb[:, kc, cB:cB + 65],
                            e[:, 512:1024],
                            start=first, stop=last,
                        )

                    def emit_norm(p, qc, oA, oB):
                        hA, hB = 2 * p, 2 * p + 1
                        q0 = qc * 512
                        denA = obuf.tile([1, 512], F32, tag="denA")
                        denB = obuf.tile([1, 512], F32, tag="denB")
                        nc.vector.tensor_copy(denA[:], oA[64:65, :])
                        nc.vector.tensor_copy(denB[:], oB[64:65, :])
                        recA = obuf.tile([1, 512], F32, tag="recA")
                        recB = obuf.tile([1, 512], F32, tag="recB")
                        nc.vector.reciprocal_approx_fast(recA[:], denA[:])
                        nc.vector.reciprocal_approx_fast(recB[:], denB[:])
                        for idx, (o_ps, rec, h, bv_sb) in enumerate((
                            (oA, recA, hA, bvA_sb),
                            (oB, recB, hB, bvB_sb),
                        )):
                            bc_sb = obuf.tile([64, 512], F32, tag=f"bcsb{idx}")
                            nc.gpsimd.partition_broadcast(bc_sb[:], rec[:])
                            o_sb = obuf.tile([64, 512], F32, tag=f"osb{idx}")
                            nc.vector.tensor_tensor(
                                out=o_sb[:], in0=o_ps[0:64, :], in1=bc_sb[:],
                                op=ALU.mult,
                            )
                            if has_bv:
                                nc.vector.tensor_scalar_add(
                                    o_sb[:], o_sb[:], bv_sb[:, p:p + 1]
                                )
                            d0 = p * 128 + (h % 2) * 64
                            nc.sync.dma_start(
                                out_d[d0:d0 + 64, q0:q0 + 512], o_sb[:]
                            )

                    def emit_scores_at(j):
                        # weave the deferred V projections ahead of the first
                        # few units' scores (chunk pair j//2 covers the keys
                        # units j and j+1 consume)
                        if j < 16 and j % 2 == 0:
                            emit_vchains(j // 2)
                        emit_scores(units[j])

                    emit_scores_at(0)
                    emit_scores_at(1)
                    o_cur = None
                    for i, u in enumerate(units):
                        p, qc, ks = u
                        if ks == 0:
                            oA = opsp.tile([65, 512], F32, tag="oA")
                            oB = opsp.tile([65, 512], F32, tag="oB")
                            o_cur = (oA, oB)
                        # PV first: it unblocks on exp(i) which completes
                        # before scores(i+2)'s ring slot frees, so the PE
                        # never head-blocks on a later dependency.
                        emit_exp_pv(u, *o_cur)
                        if i + 2 < len(units):
                            emit_scores_at(i + 2)
                        if ks == KSTEPS - 1:
                            emit_norm(p, qc, *o_cur)

            loop_cm = (
                tc.For_i(0, repeat, 1) if repeat > 1 else contextlib.nullcontext()
            )
            with loop_cm:
                _emit_body()

    nc.finalize()
    return nc


def _get_nc(fast_mask: bool, has_bqk: bool, has_bv: bool):
    key = (fast_mask, has_bqk, has_bv)
    if key not in _NC_CACHE:
        _NC_CACHE[key] = _build(*key)
    return _NC_CACHE[key]


def _prep_in_maps(x, masked_attention, Wq, bq, Wk, bk, Wv, bv):
    x = np.asarray(x, np.float32)
    mask = np.asarray(masked_attention, np.float32)
    Wq = np.asarray(Wq, np.float32)
    Wk = np.asarray(Wk, np.float32)
    Wv = np.asarray(Wv, np.float32)
    bq = np.asarray(bq, np.float32)
    bk = np.asarray(bk, np.float32)
    bv = np.asarray(bv, np.float32)

    ones = np.ones((1, 128), np.float32)
    maskb = (mask - 1.0) * 10000.0

    per_g = []
    for g in range(2):
        sl = slice(g * GDIM, (g + 1) * GDIM)
        wqT = np.ascontiguousarray(Wq[sl, :].T)
        wkT = np.ascontiguousarray(Wk[sl, :].T)
        wvT = np.ascontiguousarray(Wv[sl, :].T)
        bq_g = bq[sl].copy()
        bk_g = bk[sl].copy()
        bv_g = bv[sl].reshape(8, 64)
        bvA = np.ascontiguousarray(bv_g[0::2].T)  # [64, PAIRS]
        bvB = np.ascontiguousarray(bv_g[1::2].T)
        per_g.append((wqT, wkT, wvT, bq_g, bk_g, bvA, bvB))

    xT = [np.ascontiguousarray(x[b].T) for b in range(B)]

    in_maps = []
    for c in range(N_CORES):
        g, b = divmod(c, B)
        wqT, wkT, wvT, bq_g, bk_g, bvA, bvB = per_g[g]
        in_maps.append({
            "xT": xT[b],
            "wqT": wqT, "wkT": wkT, "wvT": wvT,
            "bq": bq_g, "bk": bk_g, "bvA": bvA, "bvB": bvB,
            "maskb": np.ascontiguousarray(maskb[b]),
            "ones": ones,
        })

    fast_mask = bool(np.all(mask == 1.0))
    has_bqk = bool(np.any(bq) or np.any(bk))
    has_bv = bool(np.any(bv))
    return in_maps, fast_mask, has_bqk, has_bv


def _gather(results):
    out = np.empty((B, L, HIDDEN), np.float32)
    for c in range(N_CORES):
        g, b = divmod(c, B)
        out[b, :, g * GDIM:(g + 1) * GDIM] = results[c]["out"].T
    return out


def kernel(x, masked_attention, Wq, bq, Wk, bk, Wv, bv):
    in_maps, fast_mask, has_bqk, has_bv = _prep_in_maps(
        x, masked_attention, Wq, bq, Wk, bk, Wv, bv
    )
    nc = _get_nc(fast_mask, has_bqk, has_bv)
    res = run_bass_kernel_spmd(nc, in_maps, core_ids=list(range(N_CORES)))
    return _gather(res.results)
